# revision 4
# baseline (speedup 1.0000x reference)
"""Trainium2 Bass kernel for nn_Digital_update (dense_mlp).

Per batch element b, user u:
    B_norm[b,u,:] = sum over 64 antennas of B[b,:,u,:]          # [.., 62]
    x = concat([D[b,u,:], B_norm[b,u,:]])                       # [64]
    h = relu(x@W1+b1); h = relu(h@W2+b2); h = relu(h@W3+b3)
    D1 = sigmoid(h@W4+b4)                                       # [2]
    out[b,u,:] = P * D1 / sum_u(D1)

Implementation: pure data-parallel over 8 NeuronCores (64 batches each),
4 groups of 16 batches per core.  B is DMA'd ant-major — partition =
(16 batches x 8 antennas), free = the contiguous (user, feat) block of
7936B — which keeps SDMA at line rate (the dominant cost: ~32MB/core).
The 64-antenna reduction runs on the TensorEngine as a block-diagonal
ones matmul (contract over the 8 resident antennas, PSUM-accumulate the
8 antenna octets).  Matmuls use float32r (1 cyc/row vs 4 for fp32;
~11 mantissa bits, plenty for the 2e-2-scale error budget); operands are
rounded to f32r for free during the cast DMAs / activation writes.
Activations stay feature-major (features on partitions, rows on the free
axis) so the MLP needs no inter-layer transposes; x^T is assembled from
per-user PE transposes of B_norm.  The per-batch user-sum normalization
is a free-axis reduce + reciprocal + broadcast multiply on DVE.
"""

import numpy as np

N_CORES = 8
BATCH, NUM_M, NUM_USER, FEAT_B = 512, 64, 32, 62
BPC = BATCH // N_CORES            # batches per core = 64
GROUP_B = 16                      # batches per group
GROUPS = BPC // GROUP_B           # 4 groups per core
ROWS_G = GROUP_B * NUM_USER       # 512 rows per group
ANT_RES = 8                       # antennas resident per B tile (partition dim)
OCTETS = NUM_M // ANT_RES         # 8 PSUM-accumulated slices
UF = NUM_USER * FEAT_B            # 1984 contiguous (user, feat) elements
NCHUNK = 4                        # 1984 = 4 x 496 matmul column chunks

PRECISION = 'fp32r'               # 'fp32r' (fast) or 'fp32' (exact, ~2.5x slower)

_CACHE = {}


def _build(precision):
    import concourse.bacc as bacc
    import concourse.tile as tile
    from concourse import mybir
    from concourse.bass import ts

    f32 = mybir.dt.float32
    f32r = mybir.dt.float32r
    AF = mybir.ActivationFunctionType
    fast = precision == 'fp32r'
    mmdt = f32r if fast else f32          # dtype of matmul-feeding tiles
    dma_eng = None                        # chosen per-tensor below

    nc = bacc.Bacc()
    Bd = nc.dram_tensor('B', [BPC, NUM_M, NUM_USER, FEAT_B], f32, kind='ExternalInput')
    Dtd = nc.dram_tensor('Dt', [2, NUM_USER, BPC], f32, kind='ExternalInput')
    W1d = nc.dram_tensor('W1p', [64, 512], f32, kind='ExternalInput')
    W2d = nc.dram_tensor('W2', [512, 512], f32, kind='ExternalInput')
    W3d = nc.dram_tensor('W3', [512, 512], f32, kind='ExternalInput')
    W4d = nc.dram_tensor('W4', [512, 2], f32, kind='ExternalInput')
    BIAS123d = nc.dram_tensor('bias123', [128, 12], f32, kind='ExternalInput')
    B4d = nc.dram_tensor('b4', [2, 1], f32, kind='ExternalInput')
    Pd = nc.dram_tensor('P', [1, 1], f32, kind='ExternalInput')
    O16d = nc.dram_tensor('ones16', [128, 16], f32, kind='ExternalInput')
    I16d = nc.dram_tensor('ident16', [16, 16], f32, kind='ExternalInput')
    Od = nc.dram_tensor('out', [2, NUM_USER, BPC], f32, kind='ExternalOutput')

    def wload(dst, src_ap):
        # weight-style load, casting to f32r on the fly when in fast mode
        if fast:
            nc.gpsimd.dma_start(out=dst, in_=src_ap)
        else:
            nc.sync.dma_start(out=dst, in_=src_ap)

    with tile.TileContext(nc) as tc:
        with (
            tc.tile_pool(name='w', bufs=1) as wpool,
            tc.tile_pool(name='bt', bufs=10) as bpool,
            tc.tile_pool(name='bn', bufs=2) as nrm,
            tc.tile_pool(name='xp', bufs=2) as xpool,
            tc.tile_pool(name='hp', bufs=2) as hpool,
            tc.tile_pool(name='sp', bufs=2) as spool,
            tc.tile_pool(name='bnps', bufs=2, space='PSUM') as rp,
            tc.tile_pool(name='pxp', bufs=2, space='PSUM') as pt,
            tc.tile_pool(name='psh', bufs=3, space='PSUM') as ph,
        ):
            w1 = wpool.tile([64, 512], mmdt)
            wload(w1, W1d[:])
            w2 = wpool.tile([128, 4, 512], mmdt)
            wload(w2, W2d[:].rearrange('(k p) m -> p k m', p=128))
            w3 = wpool.tile([128, 4, 512], mmdt)
            wload(w3, W3d[:].rearrange('(k p) m -> p k m', p=128))
            w4 = wpool.tile([128, 4, 2], mmdt)
            wload(w4, W4d[:].rearrange('(k p) c -> p k c', p=128))
            ones16 = wpool.tile([128, 16], mmdt)
            wload(ones16, O16d[:])
            ident16 = wpool.tile([16, 16], f32)
            nc.sync.dma_start(out=ident16, in_=I16d[:])
            bias123 = wpool.tile([128, 12], f32)
            nc.sync.dma_start(out=bias123, in_=BIAS123d[:])
            b4sb = wpool.tile([2, 1], f32)
            nc.sync.dma_start(out=b4sb, in_=B4d[:])
            psb = wpool.tile([2, 1], f32)
            nc.sync.dma_start(out=psb, in_=Pd[:].broadcast_to((2, 1)))

            for g in range(GROUPS):
                bsl = slice(g * GROUP_B, (g + 1) * GROUP_B)

                # ---- B loads: 8 x 1MB, partition=(16b x 8ant), 7936B runs ----
                bsrcs = []
                for a in range(OCTETS):
                    bsrc = bpool.tile([128, UF], mmdt)
                    src = Bd[bsl, a * ANT_RES:(a + 1) * ANT_RES].rearrange(
                        'b a u f -> b a (u f)')
                    if fast:
                        nc.gpsimd.dma_start(out=bsrc, in_=src)
                    else:
                        nc.sync.dma_start(out=bsrc, in_=src)
                    bsrcs.append(bsrc)

                # ---- antenna reduction on PE: bn[16 batches, (u f)] ----
                bn_sb = nrm.tile([16, UF], f32)
                for q in range(NCHUNK):
                    bn_ps = rp.tile([16, 496], f32)
                    for a in range(OCTETS):
                        nc.tensor.matmul(bn_ps[:], ones16[:], bsrcs[a][:, ts(q, 496)],
                                         start=(a == 0), stop=(a == OCTETS - 1))
                    nc.vector.tensor_copy(bn_sb[:, ts(q, 496)], bn_ps[:])

                # ---- x^T [64 feats, 512 rows], row r = u*16 + b ----
                xT = xpool.tile([64, ROWS_G], mmdt)
                nc.gpsimd.dma_start(out=xT[62:64, :], in_=Dtd[:, :, bsl])
                px = pt.tile([64, ROWS_G], f32)
                for u in range(NUM_USER):
                    nc.tensor.transpose(out=px[0:62, ts(u, GROUP_B)],
                                        in_=bn_sb[:, u * FEAT_B:(u + 1) * FEAT_B],
                                        identity=ident16[:])
                nc.vector.tensor_copy(xT[0:62, :], px[0:62, :])

                # ---- MLP, feature-major ----
                h1 = hpool.tile([128, 4, ROWS_G], mmdt)
                for m in range(4):
                    ps = ph.tile([128, ROWS_G], f32, tag='ps')
                    nc.tensor.matmul(ps[:], w1[:, ts(m, 128)], xT[:],
                                     start=True, stop=True)
                    nc.scalar.activation(out=h1[:, m, :], in_=ps[:], func=AF.Relu,
                                         bias=bias123[:, 0 + m:1 + m], scale=1.0)
                h2 = hpool.tile([128, 4, ROWS_G], mmdt)
                for m in range(4):
                    ps = ph.tile([128, ROWS_G], f32, tag='ps')
                    for k in range(4):
                        nc.tensor.matmul(ps[:], w2[:, k, ts(m, 128)], h1[:, k, :],
                                         start=(k == 0), stop=(k == 3))
                    nc.scalar.activation(out=h2[:, m, :], in_=ps[:], func=AF.Relu,
                                         bias=bias123[:, 4 + m:5 + m], scale=1.0)
                h3 = hpool.tile([128, 4, ROWS_G], mmdt)
                for m in range(4):
                    ps = ph.tile([128, ROWS_G], f32, tag='ps')
                    for k in range(4):
                        nc.tensor.matmul(ps[:], w3[:, k, ts(m, 128)], h2[:, k, :],
                                         start=(k == 0), stop=(k == 3))
                    nc.scalar.activation(out=h3[:, m, :], in_=ps[:], func=AF.Relu,
                                         bias=bias123[:, 8 + m:9 + m], scale=1.0)
                ps4 = ph.tile([2, ROWS_G], f32, tag='ps')
                for k in range(4):
                    nc.tensor.matmul(ps4[:], w4[:, k, :], h3[:, k, :],
                                     start=(k == 0), stop=(k == 3))

                # ---- sigmoid + per-batch user-sum normalization ----
                sg = spool.tile([2, NUM_USER, GROUP_B], f32)
                nc.scalar.activation(
                    out=sg[:], in_=ps4[:].rearrange('c (u b) -> c u b', u=NUM_USER),
                    func=AF.Sigmoid, bias=b4sb[:], scale=1.0)
                s2 = spool.tile([2, GROUP_B], f32)
                nc.vector.tensor_reduce(out=s2[:], in_=sg[:].rearrange('c u b -> c b u'),
                                        axis=mybir.AxisListType.X,
                                        op=mybir.AluOpType.add)
                rc = spool.tile([2, GROUP_B], f32)
                nc.vector.reciprocal(rc[:], s2[:])
                nc.vector.tensor_scalar_mul(rc[:], rc[:], psb[:])
                rbc = rc[:].unsqueeze(1).broadcast_to((2, NUM_USER, GROUP_B))
                nc.vector.tensor_mul(sg[:], sg[:], rbc)

                nc.sync.dma_start(out=Od[:, :, bsl], in_=sg[:])

    nc.finalize()
    return nc


def _get_nc(precision):
    if precision not in _CACHE:
        _CACHE[precision] = _build(precision)
    return _CACHE[precision]


def _prep_inputs(D, B, P_pow_normalized, W1, b1, W2, b2, W3, b3, W4, b4):
    f = np.float32
    D = np.asarray(D, f)
    B = np.ascontiguousarray(np.asarray(B, f))
    W1 = np.asarray(W1, f)
    # x^T rows are [B_norm(62), D(2)] while the reference x is [D(2), B_norm(62)]
    W1p = np.ascontiguousarray(np.concatenate([W1[2:64], W1[0:2]], axis=0))
    bias123 = np.empty((128, 12), f)
    for l, bb in enumerate((b1, b2, b3)):
        bb = np.asarray(bb, f)
        for m in range(4):
            bias123[:, 4 * l + m] = bb[128 * m:128 * (m + 1)]
    ones16 = np.zeros((128, 16), f)
    for j in range(16):
        ones16[8 * j:8 * (j + 1), j] = 1.0
    shared = {
        'W1p': W1p,
        'W2': np.ascontiguousarray(np.asarray(W2, f)),
        'W3': np.ascontiguousarray(np.asarray(W3, f)),
        'W4': np.ascontiguousarray(np.asarray(W4, f)),
        'bias123': bias123,
        'b4': np.asarray(b4, f).reshape(2, 1).copy(),
        'P': np.asarray(P_pow_normalized, f).reshape(1, 1).copy(),
        'ones16': ones16,
        'ident16': np.eye(16, dtype=f),
    }
    in_maps = []
    for c in range(N_CORES):
        m = dict(shared)
        m['B'] = np.ascontiguousarray(B[c * BPC:(c + 1) * BPC])
        # D transposed host-side to [c, u, b] so its DMA is contiguous
        m['Dt'] = np.ascontiguousarray(
            D[c * BPC:(c + 1) * BPC].transpose(2, 1, 0))
        in_maps.append(m)
    return in_maps


def _run(inputs, trace=False, precision=None):
    from concourse.bass_utils import run_bass_kernel_spmd
    precision = precision or PRECISION
    nc = _get_nc(precision)
    in_maps = _prep_inputs(
        D=inputs['D'], B=inputs['B'], P_pow_normalized=inputs['P_pow_normalized'],
        W1=inputs['W1'], b1=inputs['b1'], W2=inputs['W2'], b2=inputs['b2'],
        W3=inputs['W3'], b3=inputs['b3'], W4=inputs['W4'], b4=inputs['b4'])
    res = run_bass_kernel_spmd(nc, in_maps, list(range(N_CORES)), trace=trace)
    # out is [2, u, b] per core -> [b, u, 2]
    out = np.concatenate(
        [res.results[c]['out'].transpose(2, 1, 0) for c in range(N_CORES)], axis=0)
    return np.ascontiguousarray(out, np.float32), res


def kernel(D, B, P_pow_normalized, D_0, W1, b1, W2, b2, W3, b3, W4, b4):
    out, _ = _run({'D': D, 'B': B, 'P_pow_normalized': P_pow_normalized,
                   'W1': W1, 'b1': b1, 'W2': W2, 'b2': b2, 'W3': W3, 'b3': b3,
                   'W4': W4, 'b4': b4})
    return out


# revision 5
# speedup vs baseline: 1.0154x; 1.0154x over previous
"""Trainium2 Bass kernel for nn_Digital_update (dense_mlp).

Per batch element b, user u:
    B_norm[b,u,:] = sum over 64 antennas of B[b,:,u,:]          # [.., 62]
    x = concat([D[b,u,:], B_norm[b,u,:]])                       # [64]
    h = relu(x@W1+b1); h = relu(h@W2+b2); h = relu(h@W3+b3)
    D1 = sigmoid(h@W4+b4)                                       # [2]
    out[b,u,:] = P * D1 / sum_u(D1)

Implementation: pure data-parallel over 8 NeuronCores (64 batches each),
4 groups of 16 batches per core.  B is DMA'd ant-major — partition =
(16 batches x 8 antennas), free = the contiguous (user, feat) block of
7936B — which keeps SDMA at line rate (the dominant cost: ~32MB/core).
The 64-antenna reduction runs on the TensorEngine as a block-diagonal
ones matmul (contract over the 8 resident antennas, PSUM-accumulate the
8 antenna octets).  Matmuls use float32r (1 cyc/row vs 4 for fp32;
~11 mantissa bits, plenty for the 2e-2-scale error budget); operands are
rounded to f32r for free during the cast DMAs / activation writes.
Activations stay feature-major (features on partitions, rows on the free
axis) so the MLP needs no inter-layer transposes; x^T is assembled from
per-user PE transposes of B_norm.  The per-batch user-sum normalization
is a free-axis reduce + reciprocal + broadcast multiply on DVE.
"""

import numpy as np

N_CORES = 8
BATCH, NUM_M, NUM_USER, FEAT_B = 512, 64, 32, 62
BPC = BATCH // N_CORES            # batches per core = 64
GROUP_B = 16                      # batches per group
GROUPS = BPC // GROUP_B           # 4 groups per core
ROWS_G = GROUP_B * NUM_USER       # 512 rows per group
ANT_RES = 8                       # antennas resident per B tile (partition dim)
OCTETS = NUM_M // ANT_RES         # 8 PSUM-accumulated slices
UF = NUM_USER * FEAT_B            # 1984 contiguous (user, feat) elements
NCHUNK = 4                        # 1984 = 4 x 496 matmul column chunks

PRECISION = 'fp32r'               # 'fp32r' (fast) or 'fp32' (exact, ~2.5x slower)

_CACHE = {}


def _build(precision):
    import concourse.bacc as bacc
    import concourse.tile as tile
    from concourse import mybir
    from concourse.bass import ts

    f32 = mybir.dt.float32
    f32r = mybir.dt.float32r
    AF = mybir.ActivationFunctionType
    fast = precision == 'fp32r'
    mmdt = f32r if fast else f32          # dtype of matmul-feeding tiles
    # Matmul-feeding DRAM tensors are declared f32r directly (raw fp32 bits;
    # the PE truncates to f32r internally) so every load runs on the fast
    # hardware DGE instead of the gpsimd cast path.

    nc = bacc.Bacc()
    Bd = nc.dram_tensor('B', [BPC, NUM_M, NUM_USER, FEAT_B], mmdt, kind='ExternalInput')
    Dtd = nc.dram_tensor('Dt', [2, NUM_USER, BPC], mmdt, kind='ExternalInput')
    W1d = nc.dram_tensor('W1p', [64, 512], mmdt, kind='ExternalInput')
    W2d = nc.dram_tensor('W2', [512, 512], mmdt, kind='ExternalInput')
    W3d = nc.dram_tensor('W3', [512, 512], mmdt, kind='ExternalInput')
    W4d = nc.dram_tensor('W4', [512, 2], mmdt, kind='ExternalInput')
    BIAS123d = nc.dram_tensor('bias123', [128, 12], f32, kind='ExternalInput')
    B4d = nc.dram_tensor('b4', [2, 1], f32, kind='ExternalInput')
    Pd = nc.dram_tensor('P', [1, 1], f32, kind='ExternalInput')
    O16d = nc.dram_tensor('ones16', [128, 16], mmdt, kind='ExternalInput')
    I16d = nc.dram_tensor('ident16', [16, 16], f32, kind='ExternalInput')
    Od = nc.dram_tensor('out', [2, NUM_USER, BPC], f32, kind='ExternalOutput')

    def wload(dst, src_ap):
        nc.sync.dma_start(out=dst, in_=src_ap)

    with tile.TileContext(nc) as tc:
        with (
            tc.tile_pool(name='w', bufs=1) as wpool,
            tc.tile_pool(name='bt', bufs=10) as bpool,
            tc.tile_pool(name='bn', bufs=2) as nrm,
            tc.tile_pool(name='xp', bufs=2) as xpool,
            tc.tile_pool(name='hp', bufs=2) as hpool,
            tc.tile_pool(name='sp', bufs=2) as spool,
            tc.tile_pool(name='bnps', bufs=2, space='PSUM') as rp,
            tc.tile_pool(name='pxp', bufs=2, space='PSUM') as pt,
            tc.tile_pool(name='psh', bufs=3, space='PSUM') as ph,
        ):
            w1 = wpool.tile([64, 512], mmdt)
            wload(w1, W1d[:])
            w2 = wpool.tile([128, 4, 512], mmdt)
            wload(w2, W2d[:].rearrange('(k p) m -> p k m', p=128))
            w3 = wpool.tile([128, 4, 512], mmdt)
            wload(w3, W3d[:].rearrange('(k p) m -> p k m', p=128))
            w4 = wpool.tile([128, 4, 2], mmdt)
            wload(w4, W4d[:].rearrange('(k p) c -> p k c', p=128))
            ones16 = wpool.tile([128, 16], mmdt)
            wload(ones16, O16d[:])
            ident16 = wpool.tile([16, 16], f32)
            nc.sync.dma_start(out=ident16, in_=I16d[:])
            bias123 = wpool.tile([128, 12], f32)
            nc.sync.dma_start(out=bias123, in_=BIAS123d[:])
            b4sb = wpool.tile([2, 1], f32)
            nc.sync.dma_start(out=b4sb, in_=B4d[:])
            psb = wpool.tile([2, 1], f32)
            nc.sync.dma_start(out=psb, in_=Pd[:].broadcast_to((2, 1)))

            for g in range(GROUPS):
                bsl = slice(g * GROUP_B, (g + 1) * GROUP_B)

                # ---- B loads: 8 x 1MB, partition=(16b x 8ant), 7936B runs ----
                bsrcs = []
                for a in range(OCTETS):
                    bsrc = bpool.tile([128, UF], mmdt)
                    src = Bd[bsl, a * ANT_RES:(a + 1) * ANT_RES].rearrange(
                        'b a u f -> b a (u f)')
                    nc.sync.dma_start(out=bsrc, in_=src)
                    bsrcs.append(bsrc)

                # ---- antenna reduction on PE: bn[16 batches, (u f)] ----
                bn_sb = nrm.tile([16, UF], f32)
                for q in range(NCHUNK):
                    bn_ps = rp.tile([16, 496], f32)
                    for a in range(OCTETS):
                        nc.tensor.matmul(bn_ps[:], ones16[:], bsrcs[a][:, ts(q, 496)],
                                         start=(a == 0), stop=(a == OCTETS - 1))
                    nc.vector.tensor_copy(bn_sb[:, ts(q, 496)], bn_ps[:])

                # ---- x^T [64 feats, 512 rows], row r = u*16 + b ----
                xT = xpool.tile([64, ROWS_G], mmdt)
                nc.sync.dma_start(out=xT[62:64, :], in_=Dtd[:, :, bsl])
                px = pt.tile([64, ROWS_G], f32)
                for u in range(NUM_USER):
                    nc.tensor.transpose(out=px[0:62, ts(u, GROUP_B)],
                                        in_=bn_sb[:, u * FEAT_B:(u + 1) * FEAT_B],
                                        identity=ident16[:])
                nc.vector.tensor_copy(xT[0:62, :], px[0:62, :])

                # ---- MLP, feature-major ----
                h1 = hpool.tile([128, 4, ROWS_G], mmdt)
                for m in range(4):
                    ps = ph.tile([128, ROWS_G], f32, tag='ps')
                    nc.tensor.matmul(ps[:], w1[:, ts(m, 128)], xT[:],
                                     start=True, stop=True)
                    nc.scalar.activation(out=h1[:, m, :], in_=ps[:], func=AF.Relu,
                                         bias=bias123[:, 0 + m:1 + m], scale=1.0)
                h2 = hpool.tile([128, 4, ROWS_G], mmdt)
                for m in range(4):
                    ps = ph.tile([128, ROWS_G], f32, tag='ps')
                    for k in range(4):
                        nc.tensor.matmul(ps[:], w2[:, k, ts(m, 128)], h1[:, k, :],
                                         start=(k == 0), stop=(k == 3))
                    nc.scalar.activation(out=h2[:, m, :], in_=ps[:], func=AF.Relu,
                                         bias=bias123[:, 4 + m:5 + m], scale=1.0)
                h3 = hpool.tile([128, 4, ROWS_G], mmdt)
                for m in range(4):
                    ps = ph.tile([128, ROWS_G], f32, tag='ps')
                    for k in range(4):
                        nc.tensor.matmul(ps[:], w3[:, k, ts(m, 128)], h2[:, k, :],
                                         start=(k == 0), stop=(k == 3))
                    nc.scalar.activation(out=h3[:, m, :], in_=ps[:], func=AF.Relu,
                                         bias=bias123[:, 8 + m:9 + m], scale=1.0)
                ps4 = ph.tile([2, ROWS_G], f32, tag='ps')
                for k in range(4):
                    nc.tensor.matmul(ps4[:], w4[:, k, :], h3[:, k, :],
                                     start=(k == 0), stop=(k == 3))

                # ---- sigmoid + per-batch user-sum normalization ----
                sg = spool.tile([2, NUM_USER, GROUP_B], f32)
                nc.scalar.activation(
                    out=sg[:], in_=ps4[:].rearrange('c (u b) -> c u b', u=NUM_USER),
                    func=AF.Sigmoid, bias=b4sb[:], scale=1.0)
                s2 = spool.tile([2, GROUP_B], f32)
                nc.vector.tensor_reduce(out=s2[:], in_=sg[:].rearrange('c u b -> c b u'),
                                        axis=mybir.AxisListType.X,
                                        op=mybir.AluOpType.add)
                rc = spool.tile([2, GROUP_B], f32)
                nc.vector.reciprocal(rc[:], s2[:])
                nc.vector.tensor_scalar_mul(rc[:], rc[:], psb[:])
                rbc = rc[:].unsqueeze(1).broadcast_to((2, NUM_USER, GROUP_B))
                nc.vector.tensor_mul(sg[:], sg[:], rbc)

                nc.sync.dma_start(out=Od[:, :, bsl], in_=sg[:])

    nc.finalize()
    return nc


def _get_nc(precision):
    if precision not in _CACHE:
        _CACHE[precision] = _build(precision)
    return _CACHE[precision]


def _prep_inputs(D, B, P_pow_normalized, W1, b1, W2, b2, W3, b3, W4, b4):
    f = np.float32
    D = np.asarray(D, f)
    B = np.ascontiguousarray(np.asarray(B, f))
    W1 = np.asarray(W1, f)
    # x^T rows are [B_norm(62), D(2)] while the reference x is [D(2), B_norm(62)]
    W1p = np.ascontiguousarray(np.concatenate([W1[2:64], W1[0:2]], axis=0))
    bias123 = np.empty((128, 12), f)
    for l, bb in enumerate((b1, b2, b3)):
        bb = np.asarray(bb, f)
        for m in range(4):
            bias123[:, 4 * l + m] = bb[128 * m:128 * (m + 1)]
    ones16 = np.zeros((128, 16), f)
    for j in range(16):
        ones16[8 * j:8 * (j + 1), j] = 1.0
    shared = {
        'W1p': W1p,
        'W2': np.ascontiguousarray(np.asarray(W2, f)),
        'W3': np.ascontiguousarray(np.asarray(W3, f)),
        'W4': np.ascontiguousarray(np.asarray(W4, f)),
        'bias123': bias123,
        'b4': np.asarray(b4, f).reshape(2, 1).copy(),
        'P': np.asarray(P_pow_normalized, f).reshape(1, 1).copy(),
        'ones16': ones16,
        'ident16': np.eye(16, dtype=f),
    }
    in_maps = []
    for c in range(N_CORES):
        m = dict(shared)
        m['B'] = np.ascontiguousarray(B[c * BPC:(c + 1) * BPC])
        # D transposed host-side to [c, u, b] so its DMA is contiguous
        m['Dt'] = np.ascontiguousarray(
            D[c * BPC:(c + 1) * BPC].transpose(2, 1, 0))
        in_maps.append(m)
    return in_maps


def _run(inputs, trace=False, precision=None):
    from concourse.bass_utils import run_bass_kernel_spmd
    precision = precision or PRECISION
    nc = _get_nc(precision)
    in_maps = _prep_inputs(
        D=inputs['D'], B=inputs['B'], P_pow_normalized=inputs['P_pow_normalized'],
        W1=inputs['W1'], b1=inputs['b1'], W2=inputs['W2'], b2=inputs['b2'],
        W3=inputs['W3'], b3=inputs['b3'], W4=inputs['W4'], b4=inputs['b4'])
    res = run_bass_kernel_spmd(nc, in_maps, list(range(N_CORES)), trace=trace)
    # out is [2, u, b] per core -> [b, u, 2]
    out = np.concatenate(
        [res.results[c]['out'].transpose(2, 1, 0) for c in range(N_CORES)], axis=0)
    return np.ascontiguousarray(out, np.float32), res


def kernel(D, B, P_pow_normalized, D_0, W1, b1, W2, b2, W3, b3, W4, b4):
    out, _ = _run({'D': D, 'B': B, 'P_pow_normalized': P_pow_normalized,
                   'W1': W1, 'b1': b1, 'W2': W2, 'b2': b2, 'W3': W3, 'b3': b3,
                   'W4': W4, 'b4': b4})
    return out


# revision 6
# speedup vs baseline: 1.1881x; 1.1700x over previous
"""Trainium2 Bass kernel for nn_Digital_update (dense_mlp).

Per batch element b, user u:
    B_norm[b,u,:] = sum over 64 antennas of B[b,:,u,:]          # [.., 62]
    x = concat([D[b,u,:], B_norm[b,u,:]])                       # [64]
    h = relu(x@W1+b1); h = relu(h@W2+b2); h = relu(h@W3+b3)
    D1 = sigmoid(h@W4+b4)                                       # [2]
    out[b,u,:] = P * D1 / sum_u(D1)

Implementation: pure data-parallel over 8 NeuronCores (64 batches each),
4 groups of 16 batches per core.  B is DMA'd ant-major — partition =
(16 batches x 8 antennas), free = the contiguous (user, feat) block of
7936B — which keeps SDMA at line rate (the dominant cost: ~32MB/core).
The 64-antenna reduction runs on the TensorEngine as a block-diagonal
ones matmul (contract over the 8 resident antennas, PSUM-accumulate the
8 antenna octets).  Matmuls use float32r (1 cyc/row vs 4 for fp32;
~11 mantissa bits, plenty for the 2e-2-scale error budget); operands are
rounded to f32r for free during the cast DMAs / activation writes.
Activations stay feature-major (features on partitions, rows on the free
axis) so the MLP needs no inter-layer transposes; x^T is assembled from
per-user PE transposes of B_norm.  The per-batch user-sum normalization
is a free-axis reduce + reciprocal + broadcast multiply on DVE.
"""

import numpy as np

N_CORES = 8
BATCH, NUM_M, NUM_USER, FEAT_B = 512, 64, 32, 62
BPC = BATCH // N_CORES            # batches per core = 64
GROUP_B = 16                      # batches per group
GROUPS = BPC // GROUP_B           # 4 groups per core
ROWS_G = GROUP_B * NUM_USER       # 512 rows per group
PAIRS = GROUP_B // 2              # 8 B pair-tiles (2 batches x 64 ants) per group
UF = NUM_USER * FEAT_B            # 1984 contiguous (user, feat) elements
NCHUNK = 4                        # 1984 = 4 x 496 matmul column chunks

PRECISION = 'fp32r'               # 'fp32r' (fast) or 'fp32' (exact, ~2.5x slower)

_CACHE = {}


def _build(precision):
    import concourse.bacc as bacc
    import concourse.tile as tile
    from concourse import mybir
    from concourse.bass import ts

    f32 = mybir.dt.float32
    f32r = mybir.dt.float32r
    AF = mybir.ActivationFunctionType
    fast = precision == 'fp32r'
    mmdt = f32r if fast else f32          # dtype of matmul-feeding tiles
    # Matmul-feeding DRAM tensors are declared f32r directly (raw fp32 bits;
    # the PE truncates to f32r internally) so every load runs on the fast
    # hardware DGE instead of the gpsimd cast path.

    nc = bacc.Bacc()
    Bd = nc.dram_tensor('B', [BPC, NUM_M, NUM_USER, FEAT_B], mmdt, kind='ExternalInput')
    Dtd = nc.dram_tensor('Dt', [2, NUM_USER, BPC], mmdt, kind='ExternalInput')
    W1d = nc.dram_tensor('W1p', [64, 512], mmdt, kind='ExternalInput')
    W2d = nc.dram_tensor('W2', [512, 512], mmdt, kind='ExternalInput')
    W3d = nc.dram_tensor('W3', [512, 512], mmdt, kind='ExternalInput')
    W4d = nc.dram_tensor('W4', [512, 2], mmdt, kind='ExternalInput')
    BIAS123d = nc.dram_tensor('bias123', [128, 12], f32, kind='ExternalInput')
    B4d = nc.dram_tensor('b4', [2, 1], f32, kind='ExternalInput')
    Pd = nc.dram_tensor('P', [1, 1], f32, kind='ExternalInput')
    OMd = nc.dram_tensor('omask', [128, 16, 8], mmdt, kind='ExternalInput')
    I16d = nc.dram_tensor('ident16', [16, 16], f32, kind='ExternalInput')
    Od = nc.dram_tensor('out', [2, NUM_USER, BPC], f32, kind='ExternalOutput')

    def wload(dst, src_ap):
        nc.sync.dma_start(out=dst, in_=src_ap)

    with tile.TileContext(nc) as tc:
        with (
            tc.tile_pool(name='w', bufs=1) as wpool,
            tc.tile_pool(name='bt', bufs=10) as bpool,
            tc.tile_pool(name='bn', bufs=2) as nrm,
            tc.tile_pool(name='xp', bufs=2) as xpool,
            tc.tile_pool(name='hp', bufs=2) as hpool,
            tc.tile_pool(name='sp', bufs=2) as spool,
            tc.tile_pool(name='bnps', bufs=2, space='PSUM') as rp,
            tc.tile_pool(name='pxp', bufs=2, space='PSUM') as pt,
            tc.tile_pool(name='psh', bufs=3, space='PSUM') as ph,
        ):
            w1 = wpool.tile([64, 512], mmdt)
            wload(w1, W1d[:])
            w2 = wpool.tile([128, 4, 512], mmdt)
            wload(w2, W2d[:].rearrange('(k p) m -> p k m', p=128))
            w3 = wpool.tile([128, 4, 512], mmdt)
            wload(w3, W3d[:].rearrange('(k p) m -> p k m', p=128))
            w4 = wpool.tile([128, 4, 2], mmdt)
            wload(w4, W4d[:].rearrange('(k p) c -> p k c', p=128))
            omask = wpool.tile([128, 16, 8], mmdt)
            wload(omask, OMd[:])
            ident16 = wpool.tile([16, 16], f32)
            nc.sync.dma_start(out=ident16, in_=I16d[:])
            bias123 = wpool.tile([128, 12], f32)
            nc.sync.dma_start(out=bias123, in_=BIAS123d[:])
            b4sb = wpool.tile([2, 1], f32)
            nc.sync.dma_start(out=b4sb, in_=B4d[:])
            psb = wpool.tile([2, 1], f32)
            nc.sync.dma_start(out=psb, in_=Pd[:].broadcast_to((2, 1)))

            for g in range(GROUPS):
                bsl = slice(g * GROUP_B, (g + 1) * GROUP_B)

                # ---- B loads: 8 x 1MB contiguous pair-tiles (2b x 64ant) ----
                bsrcs = []
                for j in range(PAIRS):
                    bsrc = bpool.tile([128, UF], mmdt)
                    b0 = g * GROUP_B + 2 * j
                    src = Bd[b0:b0 + 2].rearrange('b a u f -> b a (u f)')
                    nc.sync.dma_start(out=bsrc, in_=src)
                    bsrcs.append(bsrc)

                # ---- antenna reduction on PE: bn[16 batches, (u f)] ----
                # pair j's mask has ones only in columns 2j, 2j+1, so all 8
                # pair-tiles accumulate into one [16, 496] PSUM chunk.
                bn_sb = nrm.tile([16, UF], f32)
                for q in range(NCHUNK):
                    bn_ps = rp.tile([16, 496], f32)
                    for j in range(PAIRS):
                        nc.tensor.matmul(bn_ps[:], omask[:, :, j],
                                         bsrcs[j][:, ts(q, 496)],
                                         start=(j == 0), stop=(j == PAIRS - 1))
                    nc.vector.tensor_copy(bn_sb[:, ts(q, 496)], bn_ps[:])

                # ---- x^T [64 feats, 512 rows], row r = u*16 + b ----
                xT = xpool.tile([64, ROWS_G], mmdt)
                nc.sync.dma_start(out=xT[62:64, :], in_=Dtd[:, :, bsl])
                px = pt.tile([64, ROWS_G], f32)
                for u in range(NUM_USER):
                    nc.tensor.transpose(out=px[0:62, ts(u, GROUP_B)],
                                        in_=bn_sb[:, u * FEAT_B:(u + 1) * FEAT_B],
                                        identity=ident16[:])
                nc.vector.tensor_copy(xT[0:62, :], px[0:62, :])

                # ---- MLP, feature-major ----
                h1 = hpool.tile([128, 4, ROWS_G], mmdt)
                for m in range(4):
                    ps = ph.tile([128, ROWS_G], f32, tag='ps')
                    nc.tensor.matmul(ps[:], w1[:, ts(m, 128)], xT[:],
                                     start=True, stop=True)
                    nc.scalar.activation(out=h1[:, m, :], in_=ps[:], func=AF.Relu,
                                         bias=bias123[:, 0 + m:1 + m], scale=1.0)
                h2 = hpool.tile([128, 4, ROWS_G], mmdt)
                for m in range(4):
                    ps = ph.tile([128, ROWS_G], f32, tag='ps')
                    for k in range(4):
                        nc.tensor.matmul(ps[:], w2[:, k, ts(m, 128)], h1[:, k, :],
                                         start=(k == 0), stop=(k == 3))
                    nc.scalar.activation(out=h2[:, m, :], in_=ps[:], func=AF.Relu,
                                         bias=bias123[:, 4 + m:5 + m], scale=1.0)
                h3 = hpool.tile([128, 4, ROWS_G], mmdt)
                for m in range(4):
                    ps = ph.tile([128, ROWS_G], f32, tag='ps')
                    for k in range(4):
                        nc.tensor.matmul(ps[:], w3[:, k, ts(m, 128)], h2[:, k, :],
                                         start=(k == 0), stop=(k == 3))
                    nc.scalar.activation(out=h3[:, m, :], in_=ps[:], func=AF.Relu,
                                         bias=bias123[:, 8 + m:9 + m], scale=1.0)
                ps4 = ph.tile([2, ROWS_G], f32, tag='ps')
                for k in range(4):
                    nc.tensor.matmul(ps4[:], w4[:, k, :], h3[:, k, :],
                                     start=(k == 0), stop=(k == 3))

                # ---- sigmoid + per-batch user-sum normalization ----
                sg = spool.tile([2, NUM_USER, GROUP_B], f32)
                nc.scalar.activation(
                    out=sg[:], in_=ps4[:].rearrange('c (u b) -> c u b', u=NUM_USER),
                    func=AF.Sigmoid, bias=b4sb[:], scale=1.0)
                s2 = spool.tile([2, GROUP_B], f32)
                nc.vector.tensor_reduce(out=s2[:], in_=sg[:].rearrange('c u b -> c b u'),
                                        axis=mybir.AxisListType.X,
                                        op=mybir.AluOpType.add)
                rc = spool.tile([2, GROUP_B], f32)
                nc.vector.reciprocal(rc[:], s2[:])
                nc.vector.tensor_scalar_mul(rc[:], rc[:], psb[:])
                rbc = rc[:].unsqueeze(1).broadcast_to((2, NUM_USER, GROUP_B))
                nc.vector.tensor_mul(sg[:], sg[:], rbc)

                nc.sync.dma_start(out=Od[:, :, bsl], in_=sg[:])

    nc.finalize()
    return nc


def _get_nc(precision):
    if precision not in _CACHE:
        _CACHE[precision] = _build(precision)
    return _CACHE[precision]


def _prep_inputs(D, B, P_pow_normalized, W1, b1, W2, b2, W3, b3, W4, b4):
    f = np.float32
    D = np.asarray(D, f)
    B = np.ascontiguousarray(np.asarray(B, f))
    W1 = np.asarray(W1, f)
    # x^T rows are [B_norm(62), D(2)] while the reference x is [D(2), B_norm(62)]
    W1p = np.ascontiguousarray(np.concatenate([W1[2:64], W1[0:2]], axis=0))
    bias123 = np.empty((128, 12), f)
    for l, bb in enumerate((b1, b2, b3)):
        bb = np.asarray(bb, f)
        for m in range(4):
            bias123[:, 4 * l + m] = bb[128 * m:128 * (m + 1)]
    omask = np.zeros((128, 16, 8), f)
    for j in range(8):
        omask[0:64, 2 * j, j] = 1.0
        omask[64:128, 2 * j + 1, j] = 1.0
    shared = {
        'W1p': W1p,
        'W2': np.ascontiguousarray(np.asarray(W2, f)),
        'W3': np.ascontiguousarray(np.asarray(W3, f)),
        'W4': np.ascontiguousarray(np.asarray(W4, f)),
        'bias123': bias123,
        'b4': np.asarray(b4, f).reshape(2, 1).copy(),
        'P': np.asarray(P_pow_normalized, f).reshape(1, 1).copy(),
        'omask': omask,
        'ident16': np.eye(16, dtype=f),
    }
    in_maps = []
    for c in range(N_CORES):
        m = dict(shared)
        m['B'] = np.ascontiguousarray(B[c * BPC:(c + 1) * BPC])
        # D transposed host-side to [c, u, b] so its DMA is contiguous
        m['Dt'] = np.ascontiguousarray(
            D[c * BPC:(c + 1) * BPC].transpose(2, 1, 0))
        in_maps.append(m)
    return in_maps


def _run(inputs, trace=False, precision=None):
    from concourse.bass_utils import run_bass_kernel_spmd
    precision = precision or PRECISION
    nc = _get_nc(precision)
    in_maps = _prep_inputs(
        D=inputs['D'], B=inputs['B'], P_pow_normalized=inputs['P_pow_normalized'],
        W1=inputs['W1'], b1=inputs['b1'], W2=inputs['W2'], b2=inputs['b2'],
        W3=inputs['W3'], b3=inputs['b3'], W4=inputs['W4'], b4=inputs['b4'])
    res = run_bass_kernel_spmd(nc, in_maps, list(range(N_CORES)), trace=trace)
    # out is [2, u, b] per core -> [b, u, 2]
    out = np.concatenate(
        [res.results[c]['out'].transpose(2, 1, 0) for c in range(N_CORES)], axis=0)
    return np.ascontiguousarray(out, np.float32), res


def kernel(D, B, P_pow_normalized, D_0, W1, b1, W2, b2, W3, b3, W4, b4):
    out, _ = _run({'D': D, 'B': B, 'P_pow_normalized': P_pow_normalized,
                   'W1': W1, 'b1': b1, 'W2': W2, 'b2': b2, 'W3': W3, 'b3': b3,
                   'W4': W4, 'b4': b4})
    return out
